# revision 3
# baseline (speedup 1.0000x reference)
"""Trainium2 Bass kernel for the dynamic-kernel ECA module.

Computation per sample:
  gap  = mean(x, axis=l)                       (c,)
  h    = gelu(gap @ w1.T + b1)                 (hidden,)
  th   = tanh(h @ w2.T + b2); delta = 2*th     scalar
  k    = (5 + clip(round(delta), -3, 3)) | 1   in {3,5,7} (delta in (-2,2))
  w    = box filter of width k in 9-tap window, 1/k weights
  y    = conv1d(gap, w) along c (zero pad 4)   (c,)
  s    = sigmoid(y)
  out  = x * s[:, None]

Sharding: pure data parallel, batch 16 -> 8 cores x 2 samples.

Memory strategy (per core, x shard = 2*512*8192 f32 = 32 MB):
  x must be read for the GAP reduction before s is known, and read again
  for the final scale.  23 of the 32 [128, 2048] tiles are kept resident
  in SBUF between the two passes; only 9 are re-read.  Traffic:
  32 (read) + 9 (re-read) + 32 (write) = 73 MB/core.

The data-dependent kernel size is handled without control flow: k only
takes values {3,5,7} with thresholds on th at 0.25 / -0.75, so the 9-tap
weight vector is a mask-blend of three host-precomputed candidates.
"""

import os
from contextlib import ExitStack

import numpy as np

import concourse.bacc as bacc
import concourse.mybir as mybir
import concourse.tile as tile
from concourse.bass_utils import run_bass_kernel_spmd

F32 = mybir.dt.float32
ALU = mybir.AluOpType
ACTF = mybir.ActivationFunctionType
AX_X = mybir.AxisListType.X

B, C, L = 16, 512, 8192
HID = 64
N_CORES = 8
BS = B // N_CORES            # samples per core = 2
CP = C // 128                # channel chunks = 4
LCH = 2048                   # l elements per tile
LP = L // LCH                # l chunks = 4
N_TILES = BS * CP * LP       # 32
N_CACHE = 22                 # tiles kept resident between the two passes


def _tile_order():
    return [(s, ci, li) for s in range(BS) for ci in range(CP) for li in range(LP)]


def _build():
    nc = bacc.Bacc("TRN2", target_bir_lowering=False, debug=False,
                   num_devices=N_CORES)

    x_d = nc.dram_tensor("x", [BS, C, L], F32, kind="ExternalInput").ap()
    w1t_d = nc.dram_tensor("w1t", [CP, 128, HID], F32, kind="ExternalInput").ap()
    b1_d = nc.dram_tensor("b1", [HID, 1], F32, kind="ExternalInput").ap()
    w2t_d = nc.dram_tensor("w2t", [HID, 1], F32, kind="ExternalInput").ap()
    b2_d = nc.dram_tensor("b2", [BS, 1], F32, kind="ExternalInput").ap()
    wks_d = nc.dram_tensor("wks", [BS, 27], F32, kind="ExternalInput").ap()
    id_d = nc.dram_tensor("ident", [128, 128], F32, kind="ExternalInput").ap()
    o_d = nc.dram_tensor("out", [BS, C, L], F32, kind="ExternalOutput").ap()

    with ExitStack() as ctx:
        tc = ctx.enter_context(tile.TileContext(nc))
        cache = ctx.enter_context(tc.tile_pool(name="cache", bufs=1))
        stream = ctx.enter_context(tc.tile_pool(name="stream", bufs=2))
        small = ctx.enter_context(tc.tile_pool(name="small", bufs=1))
        convp = ctx.enter_context(tc.tile_pool(name="convp", bufs=3))
        psum = ctx.enter_context(tc.tile_pool(name="psum", bufs=1, space="PSUM"))

        # ---- constants -------------------------------------------------
        w1t = small.tile([128, CP, HID], F32, tag="w1t")
        for i in range(CP):
            nc.sync.dma_start(out=w1t[:, i, :], in_=w1t_d[i])
        b1 = small.tile([HID, 1], F32, tag="b1")
        nc.sync.dma_start(out=b1[:], in_=b1_d[:])
        w2t = small.tile([HID, 1], F32, tag="w2t")
        nc.sync.dma_start(out=w2t[:], in_=w2t_d[:])
        b2 = small.tile([BS, 1], F32, tag="b2")
        nc.sync.dma_start(out=b2[:], in_=b2_d[:])
        wks = small.tile([BS, 27], F32, tag="wks")
        nc.sync.dma_start(out=wks[:], in_=wks_d[:])
        ident = small.tile([128, 128], F32, tag="ident")
        nc.sync.dma_start(out=ident[:], in_=id_d[:])

        # ---- pass 1: load x tiles, partial sums over l -----------------
        partials = small.tile([128, BS, CP, LP], F32, tag="partials")
        order = _tile_order()
        xt = {}
        for n, (s, ci, li) in enumerate(order):
            if n < N_CACHE:
                t = cache.tile([128, LCH], F32, tag=f"c{n}")
                xt[n] = t
            else:
                t = stream.tile([128, LCH], F32, tag="st")
            nc.sync.dma_start(
                out=t[:],
                in_=x_d[s, ci * 128:(ci + 1) * 128, li * LCH:(li + 1) * LCH])
            nc.vector.reduce_sum(out=partials[:, s, ci, li:li + 1], in_=t[:],
                                 axis=AX_X)

        gmean = small.tile([128, BS, CP], F32, tag="gmean")
        nc.vector.reduce_sum(out=gmean[:], in_=partials[:], axis=AX_X)
        nc.vector.tensor_scalar_mul(gmean[:], gmean[:], 1.0 / L)

        # ---- tiny MLP: k selection ------------------------------------
        hp = psum.tile([HID, BS], F32, tag="hp")
        for i in range(CP):
            nc.tensor.matmul(hp[:], lhsT=w1t[:, i, :], rhs=gmean[:, :, i],
                             start=(i == 0), stop=(i == CP - 1))
        h = small.tile([HID, BS], F32, tag="h")
        nc.scalar.activation(h[:], hp[:], ACTF.Gelu, bias=b1[:], scale=1.0)

        dp = psum.tile([BS, 1], F32, tag="dp")
        nc.tensor.matmul(dp[:], lhsT=h[:], rhs=w2t[:], start=True, stop=True)
        th = small.tile([BS, 1], F32, tag="th")
        nc.scalar.activation(th[:], dp[:], ACTF.Tanh, bias=b2[:], scale=1.0)

        # delta = 2*th; k = 7 iff delta >= 0.5, k = 3 iff delta < -1.5
        a = small.tile([BS, 1], F32, tag="a")
        bb = small.tile([BS, 1], F32, tag="bb")
        u = small.tile([BS, 1], F32, tag="u")
        nc.vector.tensor_scalar(out=a[:], in0=th[:], scalar1=0.25, scalar2=None,
                                op0=ALU.is_ge)
        nc.vector.tensor_scalar(out=bb[:], in0=th[:], scalar1=-0.75,
                                scalar2=None, op0=ALU.is_lt)
        nc.vector.tensor_add(u[:], a[:], bb[:])
        nc.vector.tensor_scalar(out=u[:], in0=u[:], scalar1=-1.0, scalar2=1.0,
                                op0=ALU.mult, op1=ALU.add)

        # blend the three candidate 9-tap weight vectors
        wv = small.tile([BS, 9], F32, tag="wv")
        t9 = small.tile([BS, 9], F32, tag="t9")
        nc.vector.tensor_scalar(out=wv[:], in0=wks[:, 0:9], scalar1=bb[:],
                                scalar2=None, op0=ALU.mult)
        nc.vector.tensor_scalar(out=t9[:], in0=wks[:, 9:18], scalar1=u[:],
                                scalar2=None, op0=ALU.mult)
        nc.vector.tensor_add(wv[:], wv[:], t9[:])
        nc.vector.tensor_scalar(out=t9[:], in0=wks[:, 18:27], scalar1=a[:],
                                scalar2=None, op0=ALU.mult)
        nc.vector.tensor_add(wv[:], wv[:], t9[:])

        # ---- transpose gap to sample-major, 9-tap conv, gate ----------
        gpp = psum.tile([BS, CP, 128], F32, tag="gpp")
        for i in range(CP):
            nc.tensor.matmul(gpp[:, i, :], lhsT=gmean[:, :, i],
                             rhs=ident[:], is_transpose=True,
                             start=True, stop=True)
        gp = small.tile([BS, 8 + C], F32, tag="gp")
        nc.vector.memset(gp[:], 0.0)
        nc.vector.tensor_copy(gp[:, 4:4 + C],
                              gpp[:].rearrange("s i p -> s (i p)"))

        y = small.tile([BS, C], F32, tag="y")
        nc.scalar.mul(y[:], gp[:, 0:C], wv[:, 0:1])
        for j in range(1, 9):
            tcv = convp.tile([BS, C], F32, tag="tc")
            nc.scalar.mul(tcv[:], gp[:, j:j + C], wv[:, j:j + 1])
            nc.vector.tensor_add(y[:], y[:], tcv[:])

        # sigmoid(y) = 0.5 + 0.5*tanh(y/2)  (reuses the tanh table set)
        sgr = small.tile([BS, C], F32, tag="sgr")
        nc.scalar.activation(sgr[:], y[:], ACTF.Tanh, scale=0.5)
        nc.vector.tensor_scalar(out=sgr[:], in0=sgr[:], scalar1=0.5,
                                scalar2=0.5, op0=ALU.mult, op1=ALU.add)

        # transpose gate back to channel-major [128, ci, s]
        sgp = psum.tile([128, CP, BS], F32, tag="sgp")
        for ci in range(CP):
            nc.tensor.matmul(sgp[:, ci, :],
                             lhsT=sgr[:, ci * 128:(ci + 1) * 128],
                             rhs=ident[0:BS, 0:BS], is_transpose=True,
                             start=True, stop=True)
        sg = small.tile([128, CP, BS], F32, tag="sg")
        nc.vector.tensor_copy(sg[:], sgp[:])

        # ---- pass 2: scale tiles and store ----------------------------
        for n, (s, ci, li) in enumerate(order):
            scale_ap = sg[:, ci, s:s + 1]
            if n < N_CACHE:
                t = xt[n]
            else:
                t = stream.tile([128, LCH], F32, tag="st")
                nc.sync.dma_start(
                    out=t[:],
                    in_=x_d[s, ci * 128:(ci + 1) * 128,
                            li * LCH:(li + 1) * LCH])
            if n % 2 == 0:
                nc.scalar.mul(t[:], t[:], scale_ap)
            else:
                nc.vector.tensor_scalar_mul(t[:], t[:], scale_ap)
            nc.sync.dma_start(
                out=o_d[s, ci * 128:(ci + 1) * 128, li * LCH:(li + 1) * LCH],
                in_=t[:])

    nc.compile()
    return nc


_COMPILED = None


def _get_compiled():
    global _COMPILED
    if _COMPILED is None:
        _COMPILED = _build()
    return _COMPILED


def _make_consts(w1, b1, w2, b2):
    w1 = np.asarray(w1, np.float32)
    b1 = np.asarray(b1, np.float32)
    w2 = np.asarray(w2, np.float32)
    b2 = np.asarray(b2, np.float32)
    w1t = np.ascontiguousarray(w1.T.reshape(CP, 128, HID))
    j = np.arange(9)
    cand = [(np.abs(j - 4) <= (k - 1) // 2).astype(np.float32) / np.float32(k)
            for k in (3, 5, 7)]
    wks = np.tile(np.concatenate(cand).astype(np.float32)[None, :], (BS, 1))
    return {
        "w1t": w1t,
        "b1": np.ascontiguousarray(b1.reshape(HID, 1)),
        "w2t": np.ascontiguousarray(w2.reshape(1, HID).T),
        "b2": np.full((BS, 1), float(b2.reshape(-1)[0]), np.float32),
        "wks": np.ascontiguousarray(wks),
        "ident": np.eye(128, dtype=np.float32),
    }


def kernel(x, w1, b1, w2, b2):
    x = np.asarray(x, np.float32)
    assert x.shape == (B, C, L), x.shape
    nc = _get_compiled()
    consts = _make_consts(w1, b1, w2, b2)
    in_maps = []
    for i in range(N_CORES):
        m = {"x": np.ascontiguousarray(x[i * BS:(i + 1) * BS])}
        m.update(consts)
        in_maps.append(m)
    res = run_bass_kernel_spmd(nc, in_maps, list(range(N_CORES)),
                               trace=bool(int(os.environ.get("K_TRACE", "0"))))
    out = np.concatenate([res.results[i]["out"] for i in range(N_CORES)],
                         axis=0)
    if res.exec_time_ns is not None:
        kernel.last_exec_time_ns = res.exec_time_ns
        kernel.last_mean_exec_time_ns = res.mean_exec_time_ns
    kernel.last_results = res
    return out
